# revision 8
# baseline (speedup 1.0000x reference)
"""Trainium2 Bass kernel for nn_AdaptiveExpertSystem (MoE, E=8, top-2).

Expert-parallel design: the host computes the (cheap) router on CPU and
uses it as the sharding function — each of the 8 cores receives exactly
the tokens routed to its expert (gathered, padded). The device does the
heavy math: LayerNorm (token-major), PE transposes to feature-major,
then the expert MLP (mm1 -> exact gelu -> mm2) in bf16 with weights
resident in SBUF and tokens streaming through chunk by chunk. ln_g/ln_b
are folded into w1/b1 on the host so all experts share the plain LN.
The host scatter-adds each expert's output back with the top-2 combine
weights (the unshard step).

Per-core PE work is ~2*D*I*CAP MACs (CAP ~= max tokens per expert);
weights load once and stay stationary, so the kernel sits at the bf16
matmul roofline instead of the weight-streaming bound of a
replicated-weights data-parallel layout.
"""
import numpy as np
import ml_dtypes

import concourse.bass as bass
import concourse.tile as tile
from concourse import bacc, mybir
from concourse.bass_utils import run_bass_kernel_spmd
from concourse.masks import make_identity

N_CORES = 8
B, L, D, I, E = 2, 2048, 1024, 4096, 8
NTOK = B * L
KD = D // 128       # 8  d-tiles (contraction of mm1)
NI = I // 128       # 32 i-tiles (contraction of mm2)
ND = D // 128       # 8  output d-tiles
LN_EPS = 1e-5

F32 = mybir.dt.float32
BF16 = mybir.dt.bfloat16
BF = ml_dtypes.bfloat16

_CACHE = {}


def _chunks(cap_mm):
    n = (cap_mm + 383) // 384
    base = cap_mm // n
    ws = [base] * n
    ws[-1] += cap_mm - base * n
    out, o = [], 0
    for w in ws:
        out.append((o, w))
        o += w
    return out


def build_nc(nb, cap_mm):
    """nb 128-token blocks arrive; mm runs on the first cap_mm columns."""
    chunks = _chunks(cap_mm)

    nc = bacc.Bacc(None, num_devices=N_CORES)
    xg_p = nc.declare_dram_parameter("xg", [nb, 128, D], BF16, isOutput=False)
    w1_p = nc.declare_dram_parameter("w1", [NI, 128, KD, 128], BF16,
                                     isOutput=False)
    w2_p = nc.declare_dram_parameter("w2", [ND, 128, NI, 128], BF16,
                                     isOutput=False)
    b1_p = nc.declare_dram_parameter("b1", [128, NI], F32, isOutput=False)
    out_p = nc.declare_dram_parameter("out", [ND, 128, cap_mm], BF16,
                                      isOutput=True)

    AF = mybir.ActivationFunctionType
    OP = mybir.AluOpType

    from contextlib import ExitStack
    with tile.TileContext(nc) as tc, ExitStack() as ctx:
        ep = ctx.enter_context
        consts = ep(tc.tile_pool(name="consts", bufs=1))
        xgp = ep(tc.tile_pool(name="xgp", bufs=4))
        xntp = ep(tc.tile_pool(name="xnt", bufs=1))
        w1pool = ep(tc.tile_pool(name="w1p", bufs=1))
        w2pool = ep(tc.tile_pool(name="w2p", bufs=1))
        b1pool = ep(tc.tile_pool(name="b1p", bufs=1))
        h1pool = ep(tc.tile_pool(name="h1p", bufs=1))
        h2pool = ep(tc.tile_pool(name="h2p", bufs=2))
        small = ep(tc.tile_pool(name="small", bufs=6))
        psT = ep(tc.tile_pool(name="psT", bufs=2, space="PSUM"))
        ps1 = ep(tc.tile_pool(name="ps1", bufs=3, space="PSUM"))
        ps2 = ep(tc.tile_pool(name="ps2", bufs=3, space="PSUM"))

        # ---- weight/bias DMAs (xg DMAs are interleaved per block below) ----
        b1sb = b1pool.tile([128, NI], F32)
        nc.sync.dma_start(out=b1sb, in_=b1_p[:])
        w1sb = w1pool.tile([128, NI, KD, 128], BF16)
        for i in range(NI):
            eng = nc.sync if i % 2 == 0 else nc.gpsimd
            eng.dma_start(out=w1sb[:, i], in_=w1_p[i])
        w2sb = w2pool.tile([128, ND, NI, 128], BF16)
        for d in range(ND):
            nc.scalar.dma_start(out=w2sb[:, d], in_=w2_p[d])

        ident_bf = consts.tile([128, 128], BF16)
        make_identity(nc, ident_bf)
        eps_sb = consts.tile([128, 1], F32)
        nc.vector.memset(eps_sb, LN_EPS)

        xnT = xntp.tile([128, KD, nb * 128], BF16)

        def ln_and_transpose(b):
            xt = xgp.tile([128, D], BF16, name=f"xg{b}", tag="xg")
            eng = nc.sync if b % 2 == 0 else nc.gpsimd
            eng.dma_start(out=xt[:, 0:512], in_=xg_p[b, :, 0:512])
            eng.dma_start(out=xt[:, 512:1024], in_=xg_p[b, :, 512:1024])
            xg_g = xt.rearrange("p (g d) -> p g d", g=2)
            stats = small.tile([128, 2, 6], F32, name=f"st{b}", tag=f"st{b}")
            for g in range(2):
                nc.vector.bn_stats(out=stats[:, g, :], in_=xg_g[:, g, :])
            mv = small.tile([128, 2], F32, name=f"mv{b}", tag=f"mv{b}")
            nc.vector.bn_aggr(out=mv, in_=stats)
            rstd = small.tile([128, 1], F32, name=f"rs{b}", tag=f"rs{b}")
            nc.scalar.activation(out=rstd, in_=mv[:, 1:2], func=AF.Sqrt,
                                 bias=eps_sb, scale=1.0)
            nc.vector.reciprocal(out=rstd, in_=rstd)
            nc.vector.tensor_scalar(out=xt, in0=xt,
                                    scalar1=mv[:, 0:1], scalar2=rstd,
                                    op0=OP.subtract, op1=OP.mult)
            for k in range(KD):
                pt = psT.tile([128, 128], BF16, tag="pt")
                nc.tensor.transpose(pt, xt[:, k * 128:(k + 1) * 128],
                                    ident_bf)
                nc.vector.tensor_copy(
                    out=xnT[:, k, b * 128:(b + 1) * 128], in_=pt)

        # blocks needed per chunk: emit LN/transpose just ahead of use
        done_b = 0
        for ci, (c0, w) in enumerate(chunks):
            need_b = min(nb, (c0 + w + 127) // 128)
            if ci == len(chunks) - 1:
                need_b = nb
            while done_b < need_b:
                ln_and_transpose(done_b)
                done_b += 1

            # ---- mm1 + gelu -> h1 (this chunk) ----
            h1 = h1pool.tile([128, NI, w], BF16, name=f"h1_{ci}", tag="h1",
                             bufs=1)
            for i in range(NI):
                p1 = ps1.tile([128, w], F32, tag="p1", name=f"p1_{ci}_{i}")
                for k in range(KD):
                    nc.tensor.matmul(p1, lhsT=w1sb[:, i, k],
                                     rhs=xnT[:, k, c0:c0 + w],
                                     start=(k == 0), stop=(k == KD - 1))
                nc.scalar.activation(out=h1[:, i], in_=p1, func=AF.Gelu,
                                     bias=b1sb[:, i:i + 1], scale=1.0)
            # ---- mm2 -> h2 -> DMA out ----
            for d in range(ND):
                p2 = ps2.tile([128, w], F32, tag="p2", name=f"p2_{ci}_{d}")
                for i in range(NI):
                    nc.tensor.matmul(p2, lhsT=w2sb[:, d, i], rhs=h1[:, i],
                                     start=(i == 0), stop=(i == NI - 1))
                h2 = h2pool.tile([128, w], BF16, tag="h2",
                                 name=f"h2_{ci}_{d}")
                nc.vector.tensor_copy(out=h2, in_=p2)
                eng = nc.sync if d % 2 == 0 else nc.gpsimd
                eng.dma_start(out=out_p[d][:, c0:c0 + w], in_=h2)

    nc.finalize()
    return nc


def _pack_w1(w1e):
    # [d, i] -> [i_tile, p, k, m]; d = k*128 + p, i = it*128 + m
    t = w1e.reshape(KD, 128, NI, 128)
    return np.ascontiguousarray(t.transpose(2, 1, 0, 3)).astype(BF)


def _pack_w2(w2e):
    # [i, d] -> [d_tile, p, i_tile, m]; i = it*128 + p, d = dt*128 + m
    t = w2e.reshape(NI, 128, ND, 128)
    return np.ascontiguousarray(t.transpose(2, 1, 0, 3)).astype(BF)


def kernel(**inputs) -> np.ndarray:
    x = np.asarray(inputs["hidden_states"], np.float32).reshape(NTOK, D)
    rn_g = np.asarray(inputs["rn_g"], np.float32)
    rn_b = np.asarray(inputs["rn_b"], np.float32)
    router_w = np.asarray(inputs["router_w"], np.float32)
    router_b = np.asarray(inputs["router_b"], np.float32)
    ln_g = np.asarray(inputs["ln_g"], np.float32)
    ln_b = np.asarray(inputs["ln_b"], np.float32)
    w1 = np.asarray(inputs["w1"], np.float32)
    b1 = np.asarray(inputs["b1"], np.float32)
    w2 = np.asarray(inputs["w2"], np.float32)
    b2 = np.asarray(inputs["b2"], np.float32)

    # ---- Router on host: this IS the sharding function ----
    m = x.mean(-1, keepdims=True)
    v = ((x - m) ** 2).mean(-1, keepdims=True)
    rstd = 1.0 / np.sqrt(v + LN_EPS)
    normed = (x - m) * rstd
    logits = (normed * rn_g + rn_b) @ router_w.T + router_b
    top2 = np.argsort(-logits, axis=-1, kind="stable")[:, :2]
    tv = np.take_along_axis(logits, top2, -1)
    tv = np.exp(tv - tv.max(-1, keepdims=True))
    tw = (tv / tv.sum(-1, keepdims=True)).astype(np.float32)

    idxs, wts = [], []
    for e in range(E):
        sel = (top2[:, 0] == e) | (top2[:, 1] == e)
        idx_e = np.nonzero(sel)[0]
        w_e = np.where(top2[idx_e, 0] == e, tw[idx_e, 0], tw[idx_e, 1])
        idxs.append(idx_e)
        wts.append(w_e.astype(np.float32))
    max_n = max(len(ix) for ix in idxs)
    cap_mm = max(256, ((max_n + 7) // 8) * 8)
    nb = (cap_mm + 127) // 128
    cap = nb * 128

    # ---- Per-core inputs: gathered raw tokens + this expert's weights ----
    in_maps = []
    for e in range(E):
        xg = np.zeros((cap, D), dtype=BF)
        xg[:len(idxs[e])] = x[idxs[e]].astype(BF)
        w1e = ln_g[e][:, None] * w1[e]
        b1e = b1[e] + ln_b[e] @ w1[e]
        in_maps.append({
            "xg": np.ascontiguousarray(xg.reshape(nb, 128, D)),
            "w1": _pack_w1(w1e),
            "w2": _pack_w2(w2[e]),
            "b1": np.ascontiguousarray(b1e.reshape(NI, 128).T
                                       .astype(np.float32)),
        })

    key = (nb, cap_mm)
    if key not in _CACHE:
        _CACHE[key] = build_nc(nb, cap_mm)
    nc = _CACHE[key]
    res = run_bass_kernel_spmd(nc, in_maps, core_ids=list(range(N_CORES)))

    # ---- Unshard: weighted scatter-add (top-2 combine) ----
    out = tw[:, 0:1] * b2[top2[:, 0]] + tw[:, 1:2] * b2[top2[:, 1]]
    for e in range(E):
        h2 = np.asarray(res.results[e]["out"], dtype=np.float32)
        h2 = h2.reshape(D, cap_mm).T          # [cap_mm, D]
        out[idxs[e]] += wts[e][:, None] * h2[:len(idxs[e])]
    return out.reshape(B, L, D).astype(np.float32)


# revision 11
# speedup vs baseline: 1.0192x; 1.0192x over previous
"""Trainium2 Bass kernel for nn_AdaptiveExpertSystem (MoE, E=8, top-2).

Expert-parallel design: the host computes the (cheap) router on CPU and
uses it as the sharding function — each of the 8 cores receives exactly
the tokens routed to its expert (gathered, padded). The device does the
heavy math: LayerNorm (token-major), PE transposes to feature-major,
then the expert MLP (mm1 -> exact gelu -> mm2) in bf16 with weights
resident in SBUF and tokens streaming through chunk by chunk. ln_g/ln_b
are folded into w1/b1 on the host so all experts share the plain LN.
The host scatter-adds each expert's output back with the top-2 combine
weights (the unshard step).

Per-core PE work is ~2*D*I*CAP MACs (CAP ~= max tokens per expert);
weights load once and stay stationary, so the kernel sits at the bf16
matmul roofline instead of the weight-streaming bound of a
replicated-weights data-parallel layout.
"""
import numpy as np
import ml_dtypes

import concourse.bass as bass
import concourse.tile as tile
from concourse import bacc, mybir
from concourse.bass_utils import run_bass_kernel_spmd
from concourse.masks import make_identity

N_CORES = 8
B, L, D, I, E = 2, 2048, 1024, 4096, 8
NTOK = B * L
KD = D // 128       # 8  d-tiles (contraction of mm1)
NI = I // 128       # 32 i-tiles (contraction of mm2)
ND = D // 128       # 8  output d-tiles
LN_EPS = 1e-5

F32 = mybir.dt.float32
BF16 = mybir.dt.bfloat16
BF = ml_dtypes.bfloat16

_CACHE = {}


def _chunks(cap_mm):
    n = (cap_mm + 383) // 384
    base = cap_mm // n
    ws = [base] * n
    ws[-1] += cap_mm - base * n
    out, o = [], 0
    for w in ws:
        out.append((o, w))
        o += w
    return out


def build_nc(nb, cap_mm):
    """nb 128-token blocks arrive; mm runs on the first cap_mm columns."""
    chunks = _chunks(cap_mm)

    nc = bacc.Bacc(None, num_devices=N_CORES)
    xg_p = nc.declare_dram_parameter("xg", [nb, 128, D], BF16, isOutput=False)
    w1_p = nc.declare_dram_parameter("w1", [NI, 128, KD, 128], BF16,
                                     isOutput=False)
    w2_p = nc.declare_dram_parameter("w2", [ND, 128, NI, 128], BF16,
                                     isOutput=False)
    b1_p = nc.declare_dram_parameter("b1", [128, NI], F32, isOutput=False)
    out_p = nc.declare_dram_parameter("out", [ND, 128, cap_mm], BF16,
                                      isOutput=True)

    AF = mybir.ActivationFunctionType
    OP = mybir.AluOpType

    from contextlib import ExitStack
    with tile.TileContext(nc) as tc, ExitStack() as ctx:
        ep = ctx.enter_context
        consts = ep(tc.tile_pool(name="consts", bufs=1))
        xgp = ep(tc.tile_pool(name="xgp", bufs=5))
        xntp = ep(tc.tile_pool(name="xnt", bufs=1))
        w1pool = ep(tc.tile_pool(name="w1p", bufs=1))
        w2pool = ep(tc.tile_pool(name="w2p", bufs=1))
        b1pool = ep(tc.tile_pool(name="b1p", bufs=1))
        h1pool = ep(tc.tile_pool(name="h1p", bufs=1))
        h2pool = ep(tc.tile_pool(name="h2p", bufs=2))
        small = ep(tc.tile_pool(name="small", bufs=6))
        psT = ep(tc.tile_pool(name="psT", bufs=2, space="PSUM"))
        ps1 = ep(tc.tile_pool(name="ps1", bufs=3, space="PSUM"))
        ps2 = ep(tc.tile_pool(name="ps2", bufs=3, space="PSUM"))

        # ---- weight/bias DMAs (xg DMAs are interleaved per block below) ----
        b1sb = b1pool.tile([128, NI], F32)
        nc.sync.dma_start(out=b1sb, in_=b1_p[:])
        w1sb = w1pool.tile([128, NI, KD, 128], BF16)
        for i in range(NI):
            nc.gpsimd.dma_start(out=w1sb[:, i], in_=w1_p[i])
        w2sb = w2pool.tile([128, ND, NI, 128], BF16)
        for d in range(ND):
            nc.scalar.dma_start(out=w2sb[:, d], in_=w2_p[d])

        ident_bf = consts.tile([128, 128], BF16)
        make_identity(nc, ident_bf)
        eps_sb = consts.tile([128, 1], F32)
        nc.vector.memset(eps_sb, LN_EPS)

        xnT = xntp.tile([128, KD, nb * 128], BF16)

        def ln_and_transpose(b):
            xt = xgp.tile([128, D], BF16, name=f"xg{b}", tag="xg")
            nc.sync.dma_start(out=xt[:, 0:512], in_=xg_p[b, :, 0:512])
            nc.sync.dma_start(out=xt[:, 512:1024], in_=xg_p[b, :, 512:1024])
            xg_g = xt.rearrange("p (g d) -> p g d", g=2)
            stats = small.tile([128, 2, 6], F32, name=f"st{b}", tag=f"st{b}")
            for g in range(2):
                nc.vector.bn_stats(out=stats[:, g, :], in_=xg_g[:, g, :])
            mv = small.tile([128, 2], F32, name=f"mv{b}", tag=f"mv{b}")
            nc.vector.bn_aggr(out=mv, in_=stats)
            rstd = small.tile([128, 1], F32, name=f"rs{b}", tag=f"rs{b}")
            nc.scalar.activation(out=rstd, in_=mv[:, 1:2], func=AF.Sqrt,
                                 bias=eps_sb, scale=1.0)
            nc.vector.reciprocal(out=rstd, in_=rstd)
            nc.vector.tensor_scalar(out=xt, in0=xt,
                                    scalar1=mv[:, 0:1], scalar2=rstd,
                                    op0=OP.subtract, op1=OP.mult)
            for k in range(KD):
                pt = psT.tile([128, 128], BF16, tag="pt")
                nc.tensor.transpose(pt, xt[:, k * 128:(k + 1) * 128],
                                    ident_bf)
                nc.vector.tensor_copy(
                    out=xnT[:, k, b * 128:(b + 1) * 128], in_=pt)

        # blocks needed per chunk: emit LN/transpose just ahead of use
        done_b = 0
        for ci, (c0, w) in enumerate(chunks):
            need_b = min(nb, (c0 + w + 127) // 128)
            if ci == len(chunks) - 1:
                need_b = nb
            while done_b < need_b:
                ln_and_transpose(done_b)
                done_b += 1

            # ---- mm1 + gelu -> h1 (this chunk) ----
            h1 = h1pool.tile([128, NI, w], BF16, name=f"h1_{ci}", tag="h1",
                             bufs=1)
            for i in range(NI):
                p1 = ps1.tile([128, w], F32, tag="p1", name=f"p1_{ci}_{i}")
                for k in range(KD):
                    nc.tensor.matmul(p1, lhsT=w1sb[:, i, k],
                                     rhs=xnT[:, k, c0:c0 + w],
                                     start=(k == 0), stop=(k == KD - 1))
                nc.scalar.activation(out=h1[:, i], in_=p1, func=AF.Gelu,
                                     bias=b1sb[:, i:i + 1], scale=1.0)
            # ---- mm2 -> h2 -> DMA out ----
            for d in range(ND):
                p2 = ps2.tile([128, w], F32, tag="p2", name=f"p2_{ci}_{d}")
                for i in range(NI):
                    nc.tensor.matmul(p2, lhsT=w2sb[:, d, i], rhs=h1[:, i],
                                     start=(i == 0), stop=(i == NI - 1))
                h2 = h2pool.tile([128, w], BF16, tag="h2",
                                 name=f"h2_{ci}_{d}")
                nc.vector.tensor_copy(out=h2, in_=p2)
                eng = nc.sync if d % 2 == 0 else nc.gpsimd
                eng.dma_start(out=out_p[d][:, c0:c0 + w], in_=h2)

    nc.finalize()
    return nc


def _pack_w1(w1e):
    # [d, i] -> [i_tile, p, k, m]; d = k*128 + p, i = it*128 + m
    t = w1e.reshape(KD, 128, NI, 128)
    return np.ascontiguousarray(t.transpose(2, 1, 0, 3)).astype(BF)


def _pack_w2(w2e):
    # [i, d] -> [d_tile, p, i_tile, m]; i = it*128 + p, d = dt*128 + m
    t = w2e.reshape(NI, 128, ND, 128)
    return np.ascontiguousarray(t.transpose(2, 1, 0, 3)).astype(BF)


def kernel(**inputs) -> np.ndarray:
    x = np.asarray(inputs["hidden_states"], np.float32).reshape(NTOK, D)
    rn_g = np.asarray(inputs["rn_g"], np.float32)
    rn_b = np.asarray(inputs["rn_b"], np.float32)
    router_w = np.asarray(inputs["router_w"], np.float32)
    router_b = np.asarray(inputs["router_b"], np.float32)
    ln_g = np.asarray(inputs["ln_g"], np.float32)
    ln_b = np.asarray(inputs["ln_b"], np.float32)
    w1 = np.asarray(inputs["w1"], np.float32)
    b1 = np.asarray(inputs["b1"], np.float32)
    w2 = np.asarray(inputs["w2"], np.float32)
    b2 = np.asarray(inputs["b2"], np.float32)

    # ---- Router on host: this IS the sharding function ----
    m = x.mean(-1, keepdims=True)
    v = ((x - m) ** 2).mean(-1, keepdims=True)
    rstd = 1.0 / np.sqrt(v + LN_EPS)
    normed = (x - m) * rstd
    logits = (normed * rn_g + rn_b) @ router_w.T + router_b
    top2 = np.argsort(-logits, axis=-1, kind="stable")[:, :2]
    tv = np.take_along_axis(logits, top2, -1)
    tv = np.exp(tv - tv.max(-1, keepdims=True))
    tw = (tv / tv.sum(-1, keepdims=True)).astype(np.float32)

    idxs, wts = [], []
    for e in range(E):
        sel = (top2[:, 0] == e) | (top2[:, 1] == e)
        idx_e = np.nonzero(sel)[0]
        w_e = np.where(top2[idx_e, 0] == e, tw[idx_e, 0], tw[idx_e, 1])
        idxs.append(idx_e)
        wts.append(w_e.astype(np.float32))
    max_n = max(len(ix) for ix in idxs)
    cap_mm = max(256, ((max_n + 7) // 8) * 8)
    nb = (cap_mm + 127) // 128
    cap = nb * 128

    # ---- Per-core inputs: gathered raw tokens + this expert's weights ----
    in_maps = []
    for e in range(E):
        xg = np.zeros((cap, D), dtype=BF)
        xg[:len(idxs[e])] = x[idxs[e]].astype(BF)
        w1e = ln_g[e][:, None] * w1[e]
        b1e = b1[e] + ln_b[e] @ w1[e]
        in_maps.append({
            "xg": np.ascontiguousarray(xg.reshape(nb, 128, D)),
            "w1": _pack_w1(w1e),
            "w2": _pack_w2(w2[e]),
            "b1": np.ascontiguousarray(b1e.reshape(NI, 128).T
                                       .astype(np.float32)),
        })

    key = (nb, cap_mm)
    if key not in _CACHE:
        _CACHE[key] = build_nc(nb, cap_mm)
    nc = _CACHE[key]
    res = run_bass_kernel_spmd(nc, in_maps, core_ids=list(range(N_CORES)))

    # ---- Unshard: weighted scatter-add (top-2 combine) ----
    out = tw[:, 0:1] * b2[top2[:, 0]] + tw[:, 1:2] * b2[top2[:, 1]]
    for e in range(E):
        h2 = np.asarray(res.results[e]["out"], dtype=np.float32)
        h2 = h2.reshape(D, cap_mm).T          # [cap_mm, D]
        out[idxs[e]] += wts[e][:, None] * h2[:len(idxs[e])]
    return out.reshape(B, L, D).astype(np.float32)
